# revision 6
# baseline (speedup 1.0000x reference)
"""Trainium2 Bass kernel for the DCT-CNN expert core.

Reference computation (per 512x512 single-channel image):
  1. split into 4096 non-overlapping 8x8 patches
  2. 2D DCT per patch:  c = D @ p @ D^T
  3. conv3x3(1->16, SAME) + bias + relu on each 8x8 patch image
  4. conv3x3(16->32, SAME) + bias
  5. mean over spatial (8x8), then mean over patches  -> [B, 32]

Algebraic restructuring (validated to fp32 roundoff):
  - DCT + conv1 fold into W = M1 @ (D (x) D)  [1024, 64]; the bias b1 is
    folded into the matmul by augmenting every patch with a ones-row
    (K=65), so PSUM holds  h = W p + b1h  directly.
  - relu identity:  sum_p relu(h_p) = 0.5 * (sum_p h_p + sum_p |h_p|).
    * |.| term: drained from PSUM by ScalarE (Abs activation + accum)
      and VectorE (tensor_reduce with apply_absolute_value) in parallel.
    * linear term: sum_p h_p = Waug @ q  with q = per-image sum of
      augmented patch vectors, so its contribution to the output is
      q^T (Waug @ M2e) = q^T G  with G precomputed on host. q itself is
      computed on the otherwise-idle Pool engine from the SBUF patches.
  - conv2 + spatial mean + patch mean fold into M2e [1024, 32]; the 0.5
    from the relu identity is folded into M2e and G on the host.

Device work per core (2 images = 8192 patches):
  - 64 matmuls [K=65, M=128, N=1024] bf16 -> PSUM tiles [128, 2048]
  - per tile: ACT drains cols [0:976] (Abs+accum), DVE drains cols
    [976:2048] (tensor_reduce abs) -- both engines run concurrently and
    are the pipeline bottleneck (~1.24us per 2048-col tile).
  - Pool: 8 reductions of the SBUF patch tiles -> q  (hidden)
  - tail: tiny fp32 matmuls  out = s^T M2c + q^T G + b2

Sharding: pure data parallel over images (2 per core), weights replicated.
"""
import numpy as np

import concourse.bass as bass
import concourse.bacc as bacc
import concourse.tile as tile
from concourse import mybir
from concourse.bass_utils import run_bass_kernel_spmd

N_CORES = 8
F32 = mybir.dt.float32
BF16 = mybir.dt.bfloat16

try:
    import ml_dtypes
    NP_BF16 = np.dtype(ml_dtypes.bfloat16)
except ImportError:  # pragma: no cover
    NP_BF16 = None

# ---------------------------------------------------------------- host math

def _dct_matrix(n=8):
    m = np.zeros((n, n), dtype=np.float64)
    for k in range(n):
        for t in range(n):
            if k == 0:
                m[k, t] = 1.0 / np.sqrt(n)
            else:
                m[k, t] = np.sqrt(2.0 / n) * np.cos(np.pi * k * (2 * t + 1) / (2.0 * n))
    return m


def _conv3x3_matrix(w):
    """Dense linear operator of a SAME 3x3 cross-correlation on 8x8 images.

    w: [O, I, 3, 3] -> M: [O*64, I*64] with
    flatten(conv(img))[(o,y,x)] = sum M[(o,y,x),(i,r,c)] img[i,r,c]
    """
    O, I = w.shape[0], w.shape[1]
    M = np.zeros((O, 8, 8, I, 8, 8))
    for dy in range(3):
        for dx in range(3):
            ylo, yhi = max(0, 1 - dy), min(8, 9 - dy)
            xlo, xhi = max(0, 1 - dx), min(8, 9 - dx)
            for y in range(ylo, yhi):
                for x in range(xlo, xhi):
                    M[:, y, x, :, y + dy - 1, x + dx - 1] += w[:, :, dy, dx]
    return M.reshape(O * 64, I * 64)


def _build_weights(w1, b1, w2, b2):
    """Returns (waug [65,1024] bf16-able, M2c [128,256] f32 incl 0.5,
    G [65,32] f32 incl 0.5, b2 [32] f32)."""
    D = _dct_matrix()
    KRON = np.kron(D, D)                                   # c_flat = KRON @ p_flat
    M1 = _conv3x3_matrix(w1.astype(np.float64))            # [1024, 64]
    M1K = M1 @ KRON                                        # [1024, 64]
    b1h = np.repeat(b1.astype(np.float64), 64)             # [1024]
    M2 = _conv3x3_matrix(w2.astype(np.float64))            # [2048, 1024]
    A2 = M2.reshape(32, 64, 1024).sum(axis=1)              # [32, 1024]
    M2e = A2.T / (64.0 * 4096.0)                           # [1024, 32]

    waug = np.empty((65, 1024), dtype=np.float64)
    waug[0:64] = M1K.T                                     # psum = waug^T @ p'
    waug[64] = b1h

    G = 0.5 * (waug @ M2e)                                 # [65, 32]
    M2c = np.ascontiguousarray(
        (0.5 * M2e).reshape(8, 128, 32).transpose(1, 0, 2).reshape(128, 256),
        dtype=np.float32)                                  # [128, 8*32]
    return (waug.astype(np.float32), M2c,
            np.ascontiguousarray(G, dtype=np.float32),
            b2.astype(np.float32))


# ------------------------------------------------------------- device kernel

# aux layout (f32 columns):
#   [0:256)    M2c chunks (cols 32k..32k+32 = 0.5*M2e[128k:128k+128, :])
#   [256:288)  b2 broadcast to all partitions
#   [288:320)  G (rows 0:65 meaningful)
AUXM2 = 0
AUXB2 = 256
AUXG = 288
AUXTOT = 320

ACT_COLS = 1056  # ACT drains [0:1056), DVE drains [1056:2048) of each tile
                 # (DVE also computes the q reductions, so it gets less)


def _build_nc():
    nc = bacc.Bacc("TRN2", target_bir_lowering=False, debug=False,
                   num_devices=N_CORES)
    p_d = nc.declare_dram_parameter("p", [65, 8192], BF16, isOutput=False)
    wts_d = nc.declare_dram_parameter("wts", [65, 1024], BF16, isOutput=False)
    aux_d = nc.declare_dram_parameter("aux", [128, AUXTOT], F32, isOutput=False)
    out_d = nc.declare_dram_parameter("out", [2, 32], F32, isOutput=True)

    with tile.TileContext(nc) as tc:
        with (
            tc.tile_pool(name="persist", bufs=1) as persist,
            tc.tile_pool(name="psum", bufs=2, space="PSUM") as psum,
        ):
            wts_t = persist.tile([65, 1024], BF16)
            nc.sync.dma_start(out=wts_t, in_=wts_d[:, :])

            aux_t = persist.tile([128, AUXTOT], F32)
            nc.gpsimd.dma_start(out=aux_t, in_=aux_d[:, :])

            ptiles = []
            for q in range(8):
                pt_in = persist.tile([65, 1024], BF16, tag=f"p{q}")
                eng = nc.sync if q % 2 == 0 else nc.gpsimd
                eng.dma_start(out=pt_in, in_=p_d[:, q * 1024:(q + 1) * 1024])
                ptiles.append(pt_in)

            # ACT / DVE partial |.|-sums: col 4k+j = tile (k, j)
            uacc = persist.tile([128, 32], F32)
            vacc = persist.tile([128, 32], F32)

            # DVE: per-image augmented-patch sums q from SBUF bf16 tiles
            # (2x packed mode, ~0.6us each; scheduled into DVE idle gaps)
            qtmp = persist.tile([65, 8], F32)
            q_t = persist.tile([65, 2], F32)
            for t in range(8):
                nc.vector.tensor_reduce(
                    out=qtmp[:, t:t + 1], in_=ptiles[t],
                    axis=mybir.AxisListType.X, op=mybir.AluOpType.add,
                )
            nc.vector.tensor_reduce(
                out=q_t, in_=qtmp.rearrange("p (i t) -> p i t", t=4),
                axis=mybir.AxisListType.X, op=mybir.AluOpType.add,
            )

            # Main loop: per (k, j) one PSUM tile [128, 2048] produced by
            # two 1024-col bf16 matmuls, drained concurrently by ACT
            # (Abs activation + accumulator) and DVE (tensor_reduce abs).
            for k in range(8):
                for j in range(4):
                    ps = psum.tile([128, 2048], F32, tag="ps", bufs=2)
                    for h in range(4):
                        t = 2 * j + h // 2
                        nc.tensor.matmul(
                            ps[:, 512 * h:512 * h + 512],
                            lhsT=wts_t[:, 128 * k:128 * k + 128],
                            rhs=ptiles[t][:, 512 * (h % 2):512 * (h % 2) + 512],
                            start=True, stop=True,
                        )
                    col = 4 * k + j
                    nc.scalar.activation(
                        ps[:, 0:ACT_COLS], ps[:, 0:ACT_COLS],
                        mybir.ActivationFunctionType.Abs,
                        accum_out=uacc[:, col:col + 1],
                    )
                    nc.vector.tensor_reduce(
                        out=vacc[:, col:col + 1], in_=ps[:, ACT_COLS:2048],
                        axis=mybir.AxisListType.X, op=mybir.AluOpType.add,
                        apply_absolute_value=True,
                    )

            # s[:, 2k+img] = sum_j (uacc+vacc)[:, 4k+2img+j]
            u2 = persist.tile([128, 16], F32)
            v2 = persist.tile([128, 16], F32)
            nc.vector.tensor_reduce(
                out=u2, in_=uacc.rearrange("p (g two) -> p g two", two=2),
                axis=mybir.AxisListType.X, op=mybir.AluOpType.add,
            )
            nc.vector.tensor_reduce(
                out=v2, in_=vacc.rearrange("p (g two) -> p g two", two=2),
                axis=mybir.AxisListType.X, op=mybir.AluOpType.add,
            )
            s_t = persist.tile([128, 16], F32)
            nc.vector.tensor_tensor(
                out=s_t, in0=u2, in1=v2, op=mybir.AluOpType.add,
            )

            # out[img, :] = sum_k s[:, 2k+img]^T @ M2c_k + q^T G + b2
            ps_f = psum.tile([128, 2048], F32, tag="ps", bufs=2)
            for k in range(8):
                nc.tensor.matmul(
                    ps_f[0:2, 0:32],
                    lhsT=s_t[:, 2 * k:2 * k + 2],
                    rhs=aux_t[:, AUXM2 + 32 * k:AUXM2 + 32 * k + 32],
                    start=(k == 0), stop=False,
                )
            nc.tensor.matmul(
                ps_f[0:2, 0:32],
                lhsT=q_t[:, :],
                rhs=aux_t[0:65, AUXG:AUXG + 32],
                start=False, stop=True,
            )
            out_sb = persist.tile([2, 32], F32)
            nc.vector.tensor_tensor(
                out=out_sb, in0=ps_f[0:2, 0:32], in1=aux_t[0:2, AUXB2:AUXB2 + 32],
                op=mybir.AluOpType.add,
            )
            nc.sync.dma_start(out=out_d[:, :], in_=out_sb)

    nc.compile()
    return nc


_NC_CACHE = None
TRACE = False
_last_result = None
_last_profile_dir = None


def _get_nc():
    global _NC_CACHE
    if _NC_CACHE is None:
        _NC_CACHE = _build_nc()
    return _NC_CACHE


def kernel(x, w1, b1, w2, b2):
    global _last_result
    x = np.ascontiguousarray(np.asarray(x, dtype=np.float32))
    waug, M2c, G, b2v = _build_weights(
        np.asarray(w1, np.float32), np.asarray(b1, np.float32),
        np.asarray(w2, np.float32), np.asarray(b2, np.float32))

    wts = waug.astype(NP_BF16)                             # [65, 1024]
    aux = np.zeros((128, AUXTOT), dtype=np.float32)
    aux[:, AUXM2:AUXM2 + 256] = M2c
    aux[:, AUXB2:AUXB2 + 32] = np.tile(b2v, (128, 1))
    aux[0:65, AUXG:AUXG + 32] = G

    # patches: x [16,1,512,512] -> [b, pixel(r,c), patch(i,j)] = [16, 64, 4096]
    p_all = (x.reshape(16, 64, 8, 64, 8).transpose(0, 2, 4, 1, 3)
             .reshape(16, 64, 4096))

    in_maps = []
    for c in range(N_CORES):
        pc = np.empty((65, 8192), dtype=NP_BF16)
        pc[0:64, 0:4096] = p_all[2 * c].astype(NP_BF16)
        pc[0:64, 4096:8192] = p_all[2 * c + 1].astype(NP_BF16)
        pc[64] = np.float32(1.0)
        in_maps.append({"p": pc, "wts": wts, "aux": aux})

    nc = _get_nc()
    if TRACE:
        # Local profiling path: NTFF via direct ctypes calls into the axon
        # .so (this image's antenv lacks axon_hooks; the C ABI is stable).
        import ctypes
        import contextlib
        import tempfile
        from concourse import bass2jax

        @contextlib.contextmanager
        def _ntff_hook(output_dir, device_ids):
            import jax
            jax.devices()
            lib = ctypes.CDLL("/opt/axon/libaxon_pjrt.so")
            lib.axon_start_nrt_profile.argtypes = [
                ctypes.POINTER(ctypes.c_int64), ctypes.c_size_t]
            lib.axon_start_nrt_profile.restype = ctypes.c_int64
            lib.axon_stop_nrt_profile.argtypes = [ctypes.c_char_p]
            lib.axon_stop_nrt_profile.restype = ctypes.c_int64
            if device_ids:
                ids = (ctypes.c_int64 * len(device_ids))(*device_ids)
                rc = lib.axon_start_nrt_profile(ids, len(device_ids))
            else:
                rc = lib.axon_start_nrt_profile(None, 0)
            if rc != 0:
                raise RuntimeError(f"axon_start_nrt_profile rc={rc}")
            try:
                yield
            finally:
                n = lib.axon_stop_nrt_profile(str(output_dir).encode())
                print(f"profile: {n} file(s) written to {output_dir}")

        global _last_profile_dir
        tmpdir = tempfile.mkdtemp(prefix="dctcnn_prof_")
        with _ntff_hook(tmpdir, [0]):
            results = bass2jax.run_bass_via_pjrt(nc, in_maps, n_cores=N_CORES)
        _last_profile_dir = tmpdir
        out = np.concatenate([results[c]["out"] for c in range(N_CORES)], axis=0)
        return out.astype(np.float32)
    res = run_bass_kernel_spmd(nc, in_maps, list(range(N_CORES)))
    _last_result = res
    out = np.concatenate([res.results[c]["out"] for c in range(N_CORES)], axis=0)
    return out.astype(np.float32)


# revision 12
# speedup vs baseline: 1.0529x; 1.0529x over previous
"""Trainium2 Bass kernel for the DCT-CNN expert core.

Reference computation (per 512x512 single-channel image):
  1. split into 4096 non-overlapping 8x8 patches
  2. 2D DCT per patch:  c = D @ p @ D^T
  3. conv3x3(1->16, SAME) + bias + relu on each 8x8 patch image
  4. conv3x3(16->32, SAME) + bias
  5. mean over spatial (8x8), then mean over patches  -> [B, 32]

Algebraic restructuring (validated to fp32 roundoff):
  - DCT + conv1 fold into W = M1 @ (D (x) D)  [1024, 64]; the bias b1 is
    folded into the matmul by augmenting every patch with a ones-row
    (K=65), so PSUM holds  h = W p + b1h  directly.
  - relu identity:  sum_p relu(h_p) = 0.5 * (sum_p h_p + sum_p |h_p|).
    * |.| term: drained from PSUM by ScalarE (Abs activation + accum)
      and VectorE (tensor_reduce with apply_absolute_value) in parallel.
    * linear term: sum_p h_p = Waug @ q  with q = per-image sum of
      augmented patch vectors, so its contribution to the output is
      q^T (Waug @ M2e) = q^T G  with G precomputed on host. q itself is
      computed on the otherwise-idle Pool engine from the SBUF patches.
  - conv2 + spatial mean + patch mean fold into M2e [1024, 32]; the 0.5
    from the relu identity is folded into M2e and G on the host.

Device work per core (2 images = 8192 patches):
  - 64 matmuls [K=65, M=128, N=1024] bf16 -> PSUM tiles [128, 2048]
  - per tile: ACT drains cols [0:976] (Abs+accum), DVE drains cols
    [976:2048] (tensor_reduce abs) -- both engines run concurrently and
    are the pipeline bottleneck (~1.24us per 2048-col tile).
  - Pool: 8 reductions of the SBUF patch tiles -> q  (hidden)
  - tail: tiny fp32 matmuls  out = s^T M2c + q^T G + b2

Sharding: pure data parallel over images (2 per core), weights replicated.
"""
import numpy as np

import concourse.bass as bass
import concourse.bacc as bacc
import concourse.tile as tile
from concourse import mybir
from concourse.bass_utils import run_bass_kernel_spmd

N_CORES = 8
F32 = mybir.dt.float32
BF16 = mybir.dt.bfloat16

try:
    import ml_dtypes
    NP_BF16 = np.dtype(ml_dtypes.bfloat16)
except ImportError:  # pragma: no cover
    NP_BF16 = None

# ---------------------------------------------------------------- host math

def _dct_matrix(n=8):
    m = np.zeros((n, n), dtype=np.float64)
    for k in range(n):
        for t in range(n):
            if k == 0:
                m[k, t] = 1.0 / np.sqrt(n)
            else:
                m[k, t] = np.sqrt(2.0 / n) * np.cos(np.pi * k * (2 * t + 1) / (2.0 * n))
    return m


def _conv3x3_matrix(w):
    """Dense linear operator of a SAME 3x3 cross-correlation on 8x8 images.

    w: [O, I, 3, 3] -> M: [O*64, I*64] with
    flatten(conv(img))[(o,y,x)] = sum M[(o,y,x),(i,r,c)] img[i,r,c]
    """
    O, I = w.shape[0], w.shape[1]
    M = np.zeros((O, 8, 8, I, 8, 8))
    for dy in range(3):
        for dx in range(3):
            ylo, yhi = max(0, 1 - dy), min(8, 9 - dy)
            xlo, xhi = max(0, 1 - dx), min(8, 9 - dx)
            for y in range(ylo, yhi):
                for x in range(xlo, xhi):
                    M[:, y, x, :, y + dy - 1, x + dx - 1] += w[:, :, dy, dx]
    return M.reshape(O * 64, I * 64)


def _build_weights(w1, b1, w2, b2):
    """Returns (waug [65,1024] bf16-able, M2c [128,256] f32 incl 0.5,
    G [65,32] f32 incl 0.5, b2 [32] f32)."""
    D = _dct_matrix()
    KRON = np.kron(D, D)                                   # c_flat = KRON @ p_flat
    M1 = _conv3x3_matrix(w1.astype(np.float64))            # [1024, 64]
    M1K = M1 @ KRON                                        # [1024, 64]
    b1h = np.repeat(b1.astype(np.float64), 64)             # [1024]
    M2 = _conv3x3_matrix(w2.astype(np.float64))            # [2048, 1024]
    A2 = M2.reshape(32, 64, 1024).sum(axis=1)              # [32, 1024]
    M2e = A2.T / (64.0 * 4096.0)                           # [1024, 32]

    waug = np.empty((65, 1024), dtype=np.float64)
    waug[0:64] = M1K.T                                     # psum = waug^T @ p'
    waug[64] = b1h

    G = 0.5 * (waug @ M2e)                                 # [65, 32]
    M2c = np.ascontiguousarray(
        (0.5 * M2e).reshape(8, 128, 32).transpose(1, 0, 2).reshape(128, 256),
        dtype=np.float32)                                  # [128, 8*32]
    return (waug.astype(np.float32), M2c,
            np.ascontiguousarray(G, dtype=np.float32),
            b2.astype(np.float32))


# ------------------------------------------------------------- device kernel

# aux layout (f32 columns):
#   [0:256)    M2c chunks (cols 32k..32k+32 = 0.5*M2e[128k:128k+128, :])
#   [256:288)  b2 broadcast to all partitions
#   [288:320)  G (rows 0:65 meaningful)
AUXM2 = 0
AUXB2 = 256
AUXG = 288
AUXTOT = 320

ACT_COLS = 976   # ACT drains [0:976), DVE drains [976:2048) of each tile


def _build_nc():
    nc = bacc.Bacc("TRN2", target_bir_lowering=False, debug=False,
                   num_devices=N_CORES)
    p_d = nc.declare_dram_parameter("p", [65, 8192], BF16, isOutput=False)
    wts_d = nc.declare_dram_parameter("wts", [65, 1024], BF16, isOutput=False)
    aux_d = nc.declare_dram_parameter("aux", [128, AUXTOT], F32, isOutput=False)
    out_d = nc.declare_dram_parameter("out", [2, 32], F32, isOutput=True)

    with tile.TileContext(nc) as tc:
        with (
            tc.tile_pool(name="persist", bufs=1) as persist,
            tc.tile_pool(name="psum", bufs=2, space="PSUM") as psum,
        ):
            wts_t = persist.tile([65, 1024], BF16)
            nc.sync.dma_start(out=wts_t, in_=wts_d[:, :])

            ptiles = []
            for q in range(8):
                pt_in = persist.tile([65, 1024], BF16, tag=f"p{q}")
                ptiles.append(pt_in)
            # queue order: first matmuls need p0 (sync, right after wts) and
            # p1 (gpsimd, first)
            for q in range(0, 8, 2):
                nc.sync.dma_start(out=ptiles[q], in_=p_d[:, q * 1024:(q + 1) * 1024])
            for q in range(1, 8, 2):
                nc.gpsimd.dma_start(out=ptiles[q], in_=p_d[:, q * 1024:(q + 1) * 1024])

            # ACT / DVE partial |.|-sums: col 4k+j = tile (k, j)
            uacc = persist.tile([128, 32], F32)
            vacc = persist.tile([128, 32], F32)

            # Pool: per-image augmented-patch sums q via a pairwise-add tree
            # (Pool cannot free-reduce, but tensor_tensor halving works; it
            # is otherwise idle so all of this hides under the main loop)
            q_t = persist.tile([65, 2], F32)
            qs = [persist.tile([65, 1024], F32, tag=f"qs{i}", name=f"qs{i}")
                  for i in range(2)]
            for img in range(2):
                t0 = 4 * img
                s = qs[img]
                nc.gpsimd.tensor_tensor(
                    out=s, in0=ptiles[t0], in1=ptiles[t0 + 1],
                    op=mybir.AluOpType.add)
                nc.gpsimd.tensor_tensor(
                    out=s, in0=s, in1=ptiles[t0 + 2],
                    op=mybir.AluOpType.add)
                nc.gpsimd.tensor_tensor(
                    out=s, in0=s, in1=ptiles[t0 + 3],
                    op=mybir.AluOpType.add)
                w = 1024
                while w > 1:
                    nc.gpsimd.tensor_tensor(
                        out=s[:, 0:w // 2], in0=s[:, 0:w // 2],
                        in1=s[:, w // 2:w], op=mybir.AluOpType.add)
                    w //= 2
                nc.gpsimd.tensor_copy(q_t[:, img:img + 1], s[:, 0:1])

            # Main loop: per (k, j) one PSUM tile [128, 2048] produced by
            # two 1024-col bf16 matmuls, drained concurrently by ACT
            # (Abs activation + accumulator) and DVE (tensor_reduce abs).
            for k in range(8):
                for j in range(4):
                    ps = psum.tile([128, 2048], F32, tag="ps", bufs=2)
                    for h in range(4):
                        t = 2 * j + h // 2
                        nc.tensor.matmul(
                            ps[:, 512 * h:512 * h + 512],
                            lhsT=wts_t[:, 128 * k:128 * k + 128],
                            rhs=ptiles[t][:, 512 * (h % 2):512 * (h % 2) + 512],
                            start=True, stop=True,
                        )
                    col = 4 * k + j
                    nc.scalar.activation(
                        ps[:, 0:ACT_COLS], ps[:, 0:ACT_COLS],
                        mybir.ActivationFunctionType.Abs,
                        accum_out=uacc[:, col:col + 1],
                    )
                    nc.vector.tensor_reduce(
                        out=vacc[:, col:col + 1], in_=ps[:, ACT_COLS:2048],
                        axis=mybir.AxisListType.X, op=mybir.AluOpType.add,
                        apply_absolute_value=True,
                    )

            # aux is only needed by the tail; DMA it behind the p tiles
            aux_t = persist.tile([128, AUXTOT], F32)
            nc.sync.dma_start(out=aux_t, in_=aux_d[:, :])

            # s[:, 2k+img] = sum_j (uacc+vacc)[:, 4k+2img+j]
            u2 = persist.tile([128, 16], F32)
            v2 = persist.tile([128, 16], F32)
            nc.vector.tensor_reduce(
                out=u2, in_=uacc.rearrange("p (g two) -> p g two", two=2),
                axis=mybir.AxisListType.X, op=mybir.AluOpType.add,
            )
            nc.vector.tensor_reduce(
                out=v2, in_=vacc.rearrange("p (g two) -> p g two", two=2),
                axis=mybir.AxisListType.X, op=mybir.AluOpType.add,
            )
            s_t = persist.tile([128, 16], F32)
            nc.vector.tensor_tensor(
                out=s_t, in0=u2, in1=v2, op=mybir.AluOpType.add,
            )

            # out[img, :] = sum_k s[:, 2k+img]^T @ M2c_k + q^T G + b2
            ps_f = psum.tile([128, 2048], F32, tag="ps", bufs=2)
            for k in range(8):
                nc.tensor.matmul(
                    ps_f[0:2, 0:32],
                    lhsT=s_t[:, 2 * k:2 * k + 2],
                    rhs=aux_t[:, AUXM2 + 32 * k:AUXM2 + 32 * k + 32],
                    start=(k == 0), stop=False,
                )
            nc.tensor.matmul(
                ps_f[0:2, 0:32],
                lhsT=q_t[:, :],
                rhs=aux_t[0:65, AUXG:AUXG + 32],
                start=False, stop=True,
            )
            out_sb = persist.tile([2, 32], F32)
            nc.vector.tensor_tensor(
                out=out_sb, in0=ps_f[0:2, 0:32], in1=aux_t[0:2, AUXB2:AUXB2 + 32],
                op=mybir.AluOpType.add,
            )
            nc.sync.dma_start(out=out_d[:, :], in_=out_sb)

    nc.compile()
    return nc


_NC_CACHE = None
TRACE = False
_last_result = None
_last_profile_dir = None


def _get_nc():
    global _NC_CACHE
    if _NC_CACHE is None:
        _NC_CACHE = _build_nc()
    return _NC_CACHE


def kernel(x, w1, b1, w2, b2):
    global _last_result
    x = np.ascontiguousarray(np.asarray(x, dtype=np.float32))
    waug, M2c, G, b2v = _build_weights(
        np.asarray(w1, np.float32), np.asarray(b1, np.float32),
        np.asarray(w2, np.float32), np.asarray(b2, np.float32))

    wts = waug.astype(NP_BF16)                             # [65, 1024]
    aux = np.zeros((128, AUXTOT), dtype=np.float32)
    aux[:, AUXM2:AUXM2 + 256] = M2c
    aux[:, AUXB2:AUXB2 + 32] = np.tile(b2v, (128, 1))
    aux[0:65, AUXG:AUXG + 32] = G

    # patches: x [16,1,512,512] -> [b, pixel(r,c), patch(i,j)] = [16, 64, 4096]
    p_all = (x.reshape(16, 64, 8, 64, 8).transpose(0, 2, 4, 1, 3)
             .reshape(16, 64, 4096))

    in_maps = []
    for c in range(N_CORES):
        pc = np.empty((65, 8192), dtype=NP_BF16)
        pc[0:64, 0:4096] = p_all[2 * c].astype(NP_BF16)
        pc[0:64, 4096:8192] = p_all[2 * c + 1].astype(NP_BF16)
        pc[64] = np.float32(1.0)
        in_maps.append({"p": pc, "wts": wts, "aux": aux})

    nc = _get_nc()
    if TRACE:
        # Local profiling path: NTFF via direct ctypes calls into the axon
        # .so (this image's antenv lacks axon_hooks; the C ABI is stable).
        import ctypes
        import contextlib
        import tempfile
        from concourse import bass2jax

        @contextlib.contextmanager
        def _ntff_hook(output_dir, device_ids):
            import jax
            jax.devices()
            lib = ctypes.CDLL("/opt/axon/libaxon_pjrt.so")
            lib.axon_start_nrt_profile.argtypes = [
                ctypes.POINTER(ctypes.c_int64), ctypes.c_size_t]
            lib.axon_start_nrt_profile.restype = ctypes.c_int64
            lib.axon_stop_nrt_profile.argtypes = [ctypes.c_char_p]
            lib.axon_stop_nrt_profile.restype = ctypes.c_int64
            if device_ids:
                ids = (ctypes.c_int64 * len(device_ids))(*device_ids)
                rc = lib.axon_start_nrt_profile(ids, len(device_ids))
            else:
                rc = lib.axon_start_nrt_profile(None, 0)
            if rc != 0:
                raise RuntimeError(f"axon_start_nrt_profile rc={rc}")
            try:
                yield
            finally:
                n = lib.axon_stop_nrt_profile(str(output_dir).encode())
                print(f"profile: {n} file(s) written to {output_dir}")

        global _last_profile_dir
        tmpdir = tempfile.mkdtemp(prefix="dctcnn_prof_")
        with _ntff_hook(tmpdir, [0]):
            results = bass2jax.run_bass_via_pjrt(nc, in_maps, n_cores=N_CORES)
        _last_profile_dir = tmpdir
        out = np.concatenate([results[c]["out"] for c in range(N_CORES)], axis=0)
        return out.astype(np.float32)
    res = run_bass_kernel_spmd(nc, in_maps, list(range(N_CORES)))
    _last_result = res
    out = np.concatenate([res.results[c]["out"] for c in range(N_CORES)], axis=0)
    return out.astype(np.float32)


# revision 13
# speedup vs baseline: 1.5712x; 1.4923x over previous
"""Trainium2 Bass kernel for the DCT-CNN expert core.

Reference computation (per 512x512 single-channel image):
  1. split into 4096 non-overlapping 8x8 patches
  2. 2D DCT per patch:  c = D @ p @ D^T
  3. conv3x3(1->16, SAME) + bias + relu on each 8x8 patch image
  4. conv3x3(16->32, SAME) + bias
  5. mean over spatial (8x8), then mean over patches  -> [B, 32]

Algebraic restructuring used here (validated to fp32 roundoff):
  - DCT + conv1 are both linear maps on the 64 patch pixels, so they fold
    into a single [1024, 64] matrix  W = M1 @ (D (x) D)  with bias b1
    broadcast per channel:    h1 = relu(W @ p + b1h)        [1024 per patch]
  - conv2 + spatial mean + patch mean are linear in h1, so they fold into
    a single [1024, 32] matrix applied to the per-image SUM of h1:
       out[b] = (sum_patches h1)^T @ M2e + b2
    where M2e includes the /64 spatial mean, /4096 patch mean.

Device work per core (2 images = 8192 patches):
  - 128 matmuls [K=64, M=128, N=512] (float32r) -> PSUM
  - 32 fused relu+bias+accumulate ops ([128, 2048], split ScalarE/VectorE)
  - tiny final reduction + [128,2]x[128,32] matmuls + bias add

Sharding: pure data parallel over images (2 per core), weights replicated.
"""
import numpy as np

import concourse.bass as bass
import concourse.bacc as bacc
import concourse.tile as tile
from concourse import mybir
from concourse.bass_utils import run_bass_kernel_spmd

N_CORES = 8
F32 = mybir.dt.float32
F32R = mybir.dt.float32r
BF16 = mybir.dt.bfloat16

try:
    import ml_dtypes
    NP_BF16 = np.dtype(ml_dtypes.bfloat16)
except ImportError:  # pragma: no cover
    NP_BF16 = None

# ---------------------------------------------------------------- host math

def _dct_matrix(n=8):
    m = np.zeros((n, n), dtype=np.float64)
    for k in range(n):
        for t in range(n):
            if k == 0:
                m[k, t] = 1.0 / np.sqrt(n)
            else:
                m[k, t] = np.sqrt(2.0 / n) * np.cos(np.pi * k * (2 * t + 1) / (2.0 * n))
    return m


def _conv3x3_matrix(w):
    """Dense linear operator of a SAME 3x3 cross-correlation on 8x8 images.

    w: [O, I, 3, 3] -> M: [O*64, I*64] with
    flatten(conv(img))[(o,y,x)] = sum M[(o,y,x),(i,r,c)] img[i,r,c]
    """
    O, I = w.shape[0], w.shape[1]
    M = np.zeros((O, 8, 8, I, 8, 8))
    for dy in range(3):
        for dx in range(3):
            ylo, yhi = max(0, 1 - dy), min(8, 9 - dy)
            xlo, xhi = max(0, 1 - dx), min(8, 9 - dx)
            for y in range(ylo, yhi):
                for x in range(xlo, xhi):
                    M[:, y, x, :, y + dy - 1, x + dx - 1] += w[:, :, dy, dx]
    return M.reshape(O * 64, I * 64)


def _build_weights(w1, b1, w2, b2):
    """Returns (Wt [64,1024], b1c [128,8], M2c [128,256], b2t [128,32]) f32."""
    D = _dct_matrix()
    KRON = np.kron(D, D)                                   # c_flat = KRON @ p_flat
    M1 = _conv3x3_matrix(w1.astype(np.float64))            # [1024, 64]
    M1K = M1 @ KRON                                        # [1024, 64]
    b1h = np.repeat(b1.astype(np.float64), 64)             # [1024]
    M2 = _conv3x3_matrix(w2.astype(np.float64))            # [2048, 1024]
    A2 = M2.reshape(32, 64, 1024).sum(axis=1)              # [32, 1024]
    M2e = A2.T / (64.0 * 4096.0)                           # [1024, 32]

    Wt = np.ascontiguousarray(M1K.T, dtype=np.float32)     # [64, 1024]
    b1c = np.ascontiguousarray(
        b1h.reshape(8, 128).T, dtype=np.float32)           # [128, 8]
    M2c = np.ascontiguousarray(
        M2e.reshape(8, 128, 32).transpose(1, 0, 2).reshape(128, 256),
        dtype=np.float32)                                  # [128, 8*32]
    b2t = np.ascontiguousarray(
        np.tile(b2.astype(np.float32), (128, 1)))          # [128, 32]
    return Wt, b1c, M2c, b2t


# ------------------------------------------------------------- device kernel

# wts dram param [128, 1024]: W duplicated on both partition halves
#   ([0:64) and [64:128)); consumed as float32r by the main matmuls.
# aux layout (f32 columns):
#   [0:8)      b1 chunks (col k = b1h[128k:128k+128])
#   [8:264)    M2e chunks (cols 32k..32k+32 = M2e[128k:128k+128, :])
#   [264:296)  b2 broadcast to all partitions
AUXB1 = 0
AUXM2 = 8
AUXB2 = 264
AUXTOT = 296

# relu engine assignment: ~17/32 groups on ScalarE (ACT), rest on VectorE.
_N_GROUPS = 32
_ACT_SHARE = 17


def _build_nc():
    nc = bacc.Bacc("TRN2", target_bir_lowering=False, debug=False,
                   num_devices=N_CORES)
    p_d = nc.declare_dram_parameter("p", [128, 4096], BF16, isOutput=False)
    wts_d = nc.declare_dram_parameter("wts", [128, 1024], BF16, isOutput=False)
    aux_d = nc.declare_dram_parameter("aux", [128, AUXTOT], F32, isOutput=False)
    out_d = nc.declare_dram_parameter("out", [2, 32], F32, isOutput=True)

    act_flags = [(((i + 1) * _ACT_SHARE) // _N_GROUPS) > ((i * _ACT_SHARE) // _N_GROUPS)
                 for i in range(_N_GROUPS)]

    with tile.TileContext(nc) as tc:
        with (
            tc.tile_pool(name="persist", bufs=1) as persist,
            tc.tile_pool(name="psum", bufs=2, space="PSUM") as psum,
        ):
            wts_t = persist.tile([128, 1024], BF16)
            nc.sync.dma_start(out=wts_t, in_=wts_d[:, :])

            aux_t = persist.tile([128, AUXTOT], F32)
            nc.gpsimd.dma_start(out=aux_t, in_=aux_d[:, :])

            ptiles = []
            for q in range(8):
                pt_in = persist.tile([128, 512], BF16, tag=f"p{q}")
                eng = nc.sync if q < 4 else nc.gpsimd
                eng.dma_start(
                    out=pt_in, in_=p_d[:, q * 512:(q + 1) * 512])
                ptiles.append(pt_in)

            acc_t = persist.tile([128, 64], F32)
            zeros_t = persist.tile([128, 1], F32)
            nc.vector.memset(zeros_t, 0.0)

            # Main loop: per (k, g4) produce TWO 2-bank psum groups — image
            # 0 (p partitions 0:64, PE row group 0) and image 1 (partitions
            # 64:128, row group 64). Matmuls of the two groups are
            # interleaved so consecutive MMs target different PE row groups:
            # the PE pulls the next LDWEIGHTS ahead and runs both sub-array
            # halves concurrently. bufs=2 per tag -> PE writes iteration i+1
            # while the relu engines (ScalarE for image 0, VectorE for image
            # 1, concurrently) drain iteration i.
            for k in range(8):
                b1_ap = aux_t[:, AUXB1 + k:AUXB1 + k + 1]
                for g in range(4):  # 1024-patch group
                    psA = psum.tile([128, 1024], F32, tag="psA", bufs=2)
                    psB = psum.tile([128, 1024], F32, tag="psB", bufs=2)
                    for j in range(2):
                        t = 2 * g + j
                        nc.tensor.matmul(
                            psA[:, 512 * j:512 * j + 512],
                            lhsT=wts_t[0:64, 128 * k:128 * k + 128],
                            rhs=ptiles[t][0:64, :],
                            start=True, stop=True,
                        )
                        nc.tensor.matmul(
                            psB[:, 512 * j:512 * j + 512],
                            lhsT=wts_t[64:128, 128 * k:128 * k + 128],
                            rhs=ptiles[t][64:128, :],
                            start=True, stop=True,
                        )
                    accA = acc_t[:, 8 * k + g:8 * k + g + 1]
                    if 4 * k + g == 17:
                        # lane rebalance: ScalarE costs ~1372ns/tile vs
                        # VectorE ~1291ns; 31/33 split evens the two lanes.
                        nc.vector.scalar_tensor_tensor(
                            out=psA, in0=psA, scalar=b1_ap,
                            in1=zeros_t.to_broadcast([128, 1024]),
                            op0=mybir.AluOpType.add, op1=mybir.AluOpType.max,
                            accum_out=accA,
                        )
                    else:
                        nc.scalar.activation(
                            psA, psA, mybir.ActivationFunctionType.Relu,
                            bias=b1_ap, scale=1.0, accum_out=accA,
                        )
                    # out = max(psB + b1, 0); accum_out = sum(out)
                    nc.vector.scalar_tensor_tensor(
                        out=psB, in0=psB, scalar=b1_ap,
                        in1=zeros_t.to_broadcast([128, 1024]),
                        op0=mybir.AluOpType.add, op1=mybir.AluOpType.max,
                        accum_out=acc_t[:, 8 * k + 4 + g:8 * k + 4 + g + 1],
                    )

            # s[:, 2k+img] = sum_g acc[:, 8k+4img+g]
            s_t = persist.tile([128, 16], F32)
            nc.vector.tensor_reduce(
                out=s_t,
                in_=acc_t.rearrange("p (kh g) -> p kh g", g=4),
                axis=mybir.AxisListType.X,
                op=mybir.AluOpType.add,
            )

            # out[img, :] = sum_k s[:, 2k+img]^T @ M2e_k  + b2
            ps_f = psum.tile([128, 1024], F32, tag="psA", bufs=2)
            for k in range(8):
                nc.tensor.matmul(
                    ps_f[0:2, 0:32],
                    lhsT=s_t[:, 2 * k:2 * k + 2],
                    rhs=aux_t[:, AUXM2 + 32 * k:AUXM2 + 32 * k + 32],
                    start=(k == 0), stop=(k == 7),
                )
            out_sb = persist.tile([2, 32], F32)
            nc.vector.tensor_tensor(
                out=out_sb, in0=ps_f[0:2, 0:32], in1=aux_t[0:2, AUXB2:AUXB2 + 32],
                op=mybir.AluOpType.add,
            )
            nc.sync.dma_start(out=out_d[:, :], in_=out_sb)

    nc.compile()
    return nc


_NC_CACHE = None
TRACE = False
_last_result = None
_last_profile_dir = None


def _get_nc():
    global _NC_CACHE
    if _NC_CACHE is None:
        _NC_CACHE = _build_nc()
    return _NC_CACHE


def kernel(x, w1, b1, w2, b2):
    global _last_result
    x = np.ascontiguousarray(np.asarray(x, dtype=np.float32))
    Wt, b1c, M2c, b2t = _build_weights(
        np.asarray(w1, np.float32), np.asarray(b1, np.float32),
        np.asarray(w2, np.float32), np.asarray(b2, np.float32))

    wts = np.empty((128, 1024), dtype=NP_BF16)
    wts[0:64] = Wt.astype(NP_BF16)
    wts[64:128] = wts[0:64]
    aux = np.empty((128, AUXTOT), dtype=np.float32)
    aux[:, AUXB1:AUXB1 + 8] = b1c
    aux[:, AUXM2:AUXM2 + 256] = M2c
    aux[:, AUXB2:AUXB2 + 32] = b2t

    # patches: x [16,1,512,512] -> [b, pixel(r,c), patch(i,j)] = [16, 64, 4096]
    p_all = (x.reshape(16, 64, 8, 64, 8).transpose(0, 2, 4, 1, 3)
             .reshape(16, 64, 4096).astype(NP_BF16))

    in_maps = []
    for c in range(N_CORES):
        pc = np.empty((128, 4096), dtype=NP_BF16)
        pc[0:64] = p_all[2 * c]
        pc[64:128] = p_all[2 * c + 1]
        in_maps.append({"p": pc, "wts": wts, "aux": aux})

    nc = _get_nc()
    if TRACE:
        # Local profiling path: NTFF via direct ctypes calls into the axon
        # .so (this image's antenv lacks axon_hooks; the C ABI is stable).
        import ctypes
        import contextlib
        import tempfile
        from concourse import bass2jax

        @contextlib.contextmanager
        def _ntff_hook(output_dir, device_ids):
            import jax
            jax.devices()
            lib = ctypes.CDLL("/opt/axon/libaxon_pjrt.so")
            lib.axon_start_nrt_profile.argtypes = [
                ctypes.POINTER(ctypes.c_int64), ctypes.c_size_t]
            lib.axon_start_nrt_profile.restype = ctypes.c_int64
            lib.axon_stop_nrt_profile.argtypes = [ctypes.c_char_p]
            lib.axon_stop_nrt_profile.restype = ctypes.c_int64
            if device_ids:
                ids = (ctypes.c_int64 * len(device_ids))(*device_ids)
                rc = lib.axon_start_nrt_profile(ids, len(device_ids))
            else:
                rc = lib.axon_start_nrt_profile(None, 0)
            if rc != 0:
                raise RuntimeError(f"axon_start_nrt_profile rc={rc}")
            try:
                yield
            finally:
                n = lib.axon_stop_nrt_profile(str(output_dir).encode())
                print(f"profile: {n} file(s) written to {output_dir}")

        global _last_profile_dir
        tmpdir = tempfile.mkdtemp(prefix="dctcnn_prof_")
        with _ntff_hook(tmpdir, [0]):
            results = bass2jax.run_bass_via_pjrt(nc, in_maps, n_cores=N_CORES)
        _last_profile_dir = tmpdir
        out = np.concatenate([results[c]["out"] for c in range(N_CORES)], axis=0)
        return out.astype(np.float32)
    res = run_bass_kernel_spmd(nc, in_maps, list(range(N_CORES)))
    _last_result = res
    out = np.concatenate([res.results[c]["out"] for c in range(N_CORES)], axis=0)
    return out.astype(np.float32)



# revision 14
# speedup vs baseline: 1.5992x; 1.0178x over previous
"""Trainium2 Bass kernel for the DCT-CNN expert core.

Reference computation (per 512x512 single-channel image):
  1. split into 4096 non-overlapping 8x8 patches
  2. 2D DCT per patch:  c = D @ p @ D^T
  3. conv3x3(1->16, SAME) + bias + relu on each 8x8 patch image
  4. conv3x3(16->32, SAME) + bias
  5. mean over spatial (8x8), then mean over patches  -> [B, 32]

Algebraic restructuring used here (validated to fp32 roundoff):
  - DCT + conv1 are both linear maps on the 64 patch pixels, so they fold
    into a single [1024, 64] matrix  W = M1 @ (D (x) D)  with bias b1
    broadcast per channel:    h1 = relu(W @ p + b1h)        [1024 per patch]
  - conv2 + spatial mean + patch mean are linear in h1, so they fold into
    a single [1024, 32] matrix applied to the per-image SUM of h1:
       out[b] = (sum_patches h1)^T @ M2e + b2
    where M2e includes the /64 spatial mean, /4096 patch mean.

Device work per core (2 images = 8192 patches):
  - 128 matmuls [K=64, M=128, N=512] (float32r) -> PSUM
  - 32 fused relu+bias+accumulate ops ([128, 2048], split ScalarE/VectorE)
  - tiny final reduction + [128,2]x[128,32] matmuls + bias add

Sharding: pure data parallel over images (2 per core), weights replicated.
"""
import numpy as np

import concourse.bass as bass
import concourse.bacc as bacc
import concourse.tile as tile
from concourse import mybir
from concourse.bass_utils import run_bass_kernel_spmd

N_CORES = 8
F32 = mybir.dt.float32
F32R = mybir.dt.float32r
BF16 = mybir.dt.bfloat16

try:
    import ml_dtypes
    NP_BF16 = np.dtype(ml_dtypes.bfloat16)
except ImportError:  # pragma: no cover
    NP_BF16 = None

# ---------------------------------------------------------------- host math

def _dct_matrix(n=8):
    m = np.zeros((n, n), dtype=np.float64)
    for k in range(n):
        for t in range(n):
            if k == 0:
                m[k, t] = 1.0 / np.sqrt(n)
            else:
                m[k, t] = np.sqrt(2.0 / n) * np.cos(np.pi * k * (2 * t + 1) / (2.0 * n))
    return m


def _conv3x3_matrix(w):
    """Dense linear operator of a SAME 3x3 cross-correlation on 8x8 images.

    w: [O, I, 3, 3] -> M: [O*64, I*64] with
    flatten(conv(img))[(o,y,x)] = sum M[(o,y,x),(i,r,c)] img[i,r,c]
    """
    O, I = w.shape[0], w.shape[1]
    M = np.zeros((O, 8, 8, I, 8, 8))
    for dy in range(3):
        for dx in range(3):
            ylo, yhi = max(0, 1 - dy), min(8, 9 - dy)
            xlo, xhi = max(0, 1 - dx), min(8, 9 - dx)
            for y in range(ylo, yhi):
                for x in range(xlo, xhi):
                    M[:, y, x, :, y + dy - 1, x + dx - 1] += w[:, :, dy, dx]
    return M.reshape(O * 64, I * 64)


def _build_weights(w1, b1, w2, b2):
    """Returns (Wt [64,1024], b1c [128,8], M2c [128,256], b2t [128,32]) f32."""
    D = _dct_matrix()
    KRON = np.kron(D, D)                                   # c_flat = KRON @ p_flat
    M1 = _conv3x3_matrix(w1.astype(np.float64))            # [1024, 64]
    M1K = M1 @ KRON                                        # [1024, 64]
    b1h = np.repeat(b1.astype(np.float64), 64)             # [1024]
    M2 = _conv3x3_matrix(w2.astype(np.float64))            # [2048, 1024]
    A2 = M2.reshape(32, 64, 1024).sum(axis=1)              # [32, 1024]
    M2e = A2.T / (64.0 * 4096.0)                           # [1024, 32]

    Wt = np.ascontiguousarray(M1K.T, dtype=np.float32)     # [64, 1024]
    b1c = np.ascontiguousarray(
        b1h.reshape(8, 128).T, dtype=np.float32)           # [128, 8]
    M2c = np.ascontiguousarray(
        M2e.reshape(8, 128, 32).transpose(1, 0, 2).reshape(128, 256),
        dtype=np.float32)                                  # [128, 8*32]
    b2t = np.ascontiguousarray(
        np.tile(b2.astype(np.float32), (128, 1)))          # [128, 32]
    return Wt, b1c, M2c, b2t


# ------------------------------------------------------------- device kernel

# wts dram param [128, 1024]: W duplicated on both partition halves
#   ([0:64) and [64:128)); consumed as float32r by the main matmuls.
# aux layout (f32 columns):
#   [0:8)      b1 chunks (col k = b1h[128k:128k+128])
#   [8:264)    M2e chunks (cols 32k..32k+32 = M2e[128k:128k+128, :])
#   [264:296)  b2 broadcast to all partitions
AUXB1 = 0
AUXM2 = 8
AUXB2 = 264
AUXTOT = 296

# relu engine assignment: ~17/32 groups on ScalarE (ACT), rest on VectorE.
_N_GROUPS = 32
_ACT_SHARE = 17


def _build_nc():
    nc = bacc.Bacc("TRN2", target_bir_lowering=False, debug=False,
                   num_devices=N_CORES)
    p_d = nc.declare_dram_parameter("p", [128, 4096], BF16, isOutput=False)
    wts_d = nc.declare_dram_parameter("wts", [128, 1024], BF16, isOutput=False)
    aux_d = nc.declare_dram_parameter("aux", [128, AUXTOT], F32, isOutput=False)
    out_d = nc.declare_dram_parameter("out", [2, 32], F32, isOutput=True)

    act_flags = [(((i + 1) * _ACT_SHARE) // _N_GROUPS) > ((i * _ACT_SHARE) // _N_GROUPS)
                 for i in range(_N_GROUPS)]

    with tile.TileContext(nc) as tc:
        with (
            tc.tile_pool(name="persist", bufs=1) as persist,
            tc.tile_pool(name="psum", bufs=2, space="PSUM") as psum,
        ):
            wts_t = persist.tile([128, 1024], BF16)
            nc.sync.dma_start(out=wts_t, in_=wts_d[:, :])

            aux_t = persist.tile([128, AUXTOT], F32)
            nc.gpsimd.dma_start(out=aux_t, in_=aux_d[:, :])

            ptiles = []
            for q in range(8):
                pt_in = persist.tile([128, 512], BF16, tag=f"p{q}")
                ptiles.append(pt_in)
            # interleave queues so tiles land in consumption order:
            # sync: wts, p0, p2, ...; gpsimd: aux, p1, p3, ...
            for q in range(0, 8, 2):
                nc.sync.dma_start(
                    out=ptiles[q], in_=p_d[:, q * 512:(q + 1) * 512])
            for q in range(1, 8, 2):
                nc.gpsimd.dma_start(
                    out=ptiles[q], in_=p_d[:, q * 512:(q + 1) * 512])

            acc_t = persist.tile([128, 64], F32)
            zeros_t = persist.tile([128, 1], F32)
            nc.vector.memset(zeros_t, 0.0)

            # Main loop: per (k, g4) produce TWO 2-bank psum groups — image
            # 0 (p partitions 0:64, PE row group 0) and image 1 (partitions
            # 64:128, row group 64). Matmuls of the two groups are
            # interleaved so consecutive MMs target different PE row groups:
            # the PE pulls the next LDWEIGHTS ahead and runs both sub-array
            # halves concurrently. bufs=2 per tag -> PE writes iteration i+1
            # while the relu engines (ScalarE for image 0, VectorE for image
            # 1, concurrently) drain iteration i.
            for k in range(8):
                b1_ap = aux_t[:, AUXB1 + k:AUXB1 + k + 1]
                for g in range(4):  # 1024-patch group
                    psA = psum.tile([128, 1024], F32, tag="psA", bufs=2)
                    psB = psum.tile([128, 1024], F32, tag="psB", bufs=2)
                    for j in range(2):
                        t = 2 * g + j
                        nc.tensor.matmul(
                            psA[:, 512 * j:512 * j + 512],
                            lhsT=wts_t[0:64, 128 * k:128 * k + 128],
                            rhs=ptiles[t][0:64, :],
                            start=True, stop=True,
                        )
                        nc.tensor.matmul(
                            psB[:, 512 * j:512 * j + 512],
                            lhsT=wts_t[64:128, 128 * k:128 * k + 128],
                            rhs=ptiles[t][64:128, :],
                            start=True, stop=True,
                        )
                    accA = acc_t[:, 8 * k + g:8 * k + g + 1]
                    if 4 * k + g == 17:
                        # lane rebalance: ScalarE costs ~1372ns/tile vs
                        # VectorE ~1291ns; 31/33 split evens the two lanes.
                        nc.vector.scalar_tensor_tensor(
                            out=psA, in0=psA, scalar=b1_ap,
                            in1=zeros_t.to_broadcast([128, 1024]),
                            op0=mybir.AluOpType.add, op1=mybir.AluOpType.max,
                            accum_out=accA,
                        )
                    else:
                        nc.scalar.activation(
                            psA, psA, mybir.ActivationFunctionType.Relu,
                            bias=b1_ap, scale=1.0, accum_out=accA,
                        )
                    # out = max(psB + b1, 0); accum_out = sum(out)
                    nc.vector.scalar_tensor_tensor(
                        out=psB, in0=psB, scalar=b1_ap,
                        in1=zeros_t.to_broadcast([128, 1024]),
                        op0=mybir.AluOpType.add, op1=mybir.AluOpType.max,
                        accum_out=acc_t[:, 8 * k + 4 + g:8 * k + 4 + g + 1],
                    )

            # s[:, 2k+img] = sum_g acc[:, 8k+4img+g]
            s_t = persist.tile([128, 16], F32)
            nc.vector.tensor_reduce(
                out=s_t,
                in_=acc_t.rearrange("p (kh g) -> p kh g", g=4),
                axis=mybir.AxisListType.X,
                op=mybir.AluOpType.add,
            )

            # out[img, :] = sum_k s[:, 2k+img]^T @ M2e_k  + b2
            ps_f = psum.tile([128, 1024], F32, tag="psA", bufs=2)
            for k in range(8):
                nc.tensor.matmul(
                    ps_f[0:2, 0:32],
                    lhsT=s_t[:, 2 * k:2 * k + 2],
                    rhs=aux_t[:, AUXM2 + 32 * k:AUXM2 + 32 * k + 32],
                    start=(k == 0), stop=(k == 7),
                )
            out_sb = persist.tile([2, 32], F32)
            nc.vector.tensor_tensor(
                out=out_sb, in0=ps_f[0:2, 0:32], in1=aux_t[0:2, AUXB2:AUXB2 + 32],
                op=mybir.AluOpType.add,
            )
            nc.sync.dma_start(out=out_d[:, :], in_=out_sb)

    nc.compile()
    return nc


_NC_CACHE = None
TRACE = False
_last_result = None
_last_profile_dir = None


def _get_nc():
    global _NC_CACHE
    if _NC_CACHE is None:
        _NC_CACHE = _build_nc()
    return _NC_CACHE


def kernel(x, w1, b1, w2, b2):
    global _last_result
    x = np.ascontiguousarray(np.asarray(x, dtype=np.float32))
    Wt, b1c, M2c, b2t = _build_weights(
        np.asarray(w1, np.float32), np.asarray(b1, np.float32),
        np.asarray(w2, np.float32), np.asarray(b2, np.float32))

    wts = np.empty((128, 1024), dtype=NP_BF16)
    wts[0:64] = Wt.astype(NP_BF16)
    wts[64:128] = wts[0:64]
    aux = np.empty((128, AUXTOT), dtype=np.float32)
    aux[:, AUXB1:AUXB1 + 8] = b1c
    aux[:, AUXM2:AUXM2 + 256] = M2c
    aux[:, AUXB2:AUXB2 + 32] = b2t

    # patches: x [16,1,512,512] -> [b, pixel(r,c), patch(i,j)] = [16, 64, 4096]
    p_all = (x.reshape(16, 64, 8, 64, 8).transpose(0, 2, 4, 1, 3)
             .reshape(16, 64, 4096).astype(NP_BF16))

    in_maps = []
    for c in range(N_CORES):
        pc = np.empty((128, 4096), dtype=NP_BF16)
        pc[0:64] = p_all[2 * c]
        pc[64:128] = p_all[2 * c + 1]
        in_maps.append({"p": pc, "wts": wts, "aux": aux})

    nc = _get_nc()
    if TRACE:
        # Local profiling path: NTFF via direct ctypes calls into the axon
        # .so (this image's antenv lacks axon_hooks; the C ABI is stable).
        import ctypes
        import contextlib
        import tempfile
        from concourse import bass2jax

        @contextlib.contextmanager
        def _ntff_hook(output_dir, device_ids):
            import jax
            jax.devices()
            lib = ctypes.CDLL("/opt/axon/libaxon_pjrt.so")
            lib.axon_start_nrt_profile.argtypes = [
                ctypes.POINTER(ctypes.c_int64), ctypes.c_size_t]
            lib.axon_start_nrt_profile.restype = ctypes.c_int64
            lib.axon_stop_nrt_profile.argtypes = [ctypes.c_char_p]
            lib.axon_stop_nrt_profile.restype = ctypes.c_int64
            if device_ids:
                ids = (ctypes.c_int64 * len(device_ids))(*device_ids)
                rc = lib.axon_start_nrt_profile(ids, len(device_ids))
            else:
                rc = lib.axon_start_nrt_profile(None, 0)
            if rc != 0:
                raise RuntimeError(f"axon_start_nrt_profile rc={rc}")
            try:
                yield
            finally:
                n = lib.axon_stop_nrt_profile(str(output_dir).encode())
                print(f"profile: {n} file(s) written to {output_dir}")

        global _last_profile_dir
        tmpdir = tempfile.mkdtemp(prefix="dctcnn_prof_")
        with _ntff_hook(tmpdir, [0]):
            results = bass2jax.run_bass_via_pjrt(nc, in_maps, n_cores=N_CORES)
        _last_profile_dir = tmpdir
        out = np.concatenate([results[c]["out"] for c in range(N_CORES)], axis=0)
        return out.astype(np.float32)
    res = run_bass_kernel_spmd(nc, in_maps, list(range(N_CORES)))
    _last_result = res
    out = np.concatenate([res.results[c]["out"] for c in range(N_CORES)], axis=0)
    return out.astype(np.float32)

